# revision 6
# baseline (speedup 1.0000x reference)
"""Trainium2 Bass kernel for MultiHeadSelfAttention (RMSNorm + QKV + causal SDPA + out-proj).

Sharding: 8 cores = batch(2) x head-groups(4); each core does one batch element
and 4 heads (512-wide slice); host sums the 4 partial out-projections per batch.

v3 design (cost-model driven):
  - QKV projection in fp8e4m3 DoubleRow matmuls (K=256/instr, 0.5 cyc/row) with
    3-term error compensation: W*x ~ Whi*xhi + Whi*xlo + Wlo*xhi.  Host ships
    hi/lo splits pre-scaled (x*16, w*64) so fp8 subnormals never bite; the
    1/1024 product scale is folded into the per-token RMS-norm reciprocal.
  - x8 fully SBUF-resident; q/k/v stored bf16 in SBUF (no DRAM roundtrip).
  - RMS stats computed once up front (single Sqrt -> only 2 ACT table loads).
  - Reduction matmuls (sum-of-squares, softmax z) output on PARTITIONS
    (ap_size=1) instead of a [1,N] row: near-free on the PE.
  - Out-projection transposed to [t, d_out]; per-head 1/z applied to y.
  - Engines are in-order, so attention is emitted as head PAIRS with the
    next chunk's QKV DoubleRow groups and the previous chunk's out-proj
    matmuls WOVEN between each score and its dependent exp consumers.
  - Causal mask multiplies run on the idle Pool engine; latency-critical
    small DMAs ride the SP queue (ACT queue stays pure exp).
"""

import sys

sys.path.insert(0, '/opt/trn_rl_repo')

from collections import deque

import numpy as np
import ml_dtypes

import concourse.bass as bass  # noqa: F401  (import order matters)
from concourse import bacc
import concourse.mybir as mybir
import concourse.tile as tile
from concourse.bass_utils import run_bass_kernel_spmd

B, T, D = 2, 2048, 2048
H_LOC, DH = 4, 128
EL = H_LOC * DH            # 512: local q/k/v width
NT = T // 128              # 16 t-tiles
CH = 512                   # token chunk
NCH = T // CH              # 4 chunks
QT = CH // 128             # 4 q-tiles per chunk
NKK = D // 256             # 8 double-k-tiles
EPS = 1e-6
XS, WS = 16.0, 64.0        # fp8 pre-scales
PS = XS * WS               # 1024: product scale
F32 = mybir.dt.float32
BF16 = mybir.dt.bfloat16
FP8 = mybir.dt.float8e4
DR = mybir.MatmulPerfMode.DoubleRow
MULT = mybir.AluOpType.mult
EXP = mybir.ActivationFunctionType.Exp
SQRT = mybir.ActivationFunctionType.Sqrt
E4NP = ml_dtypes.float8_e4m3
BFNP = ml_dtypes.bfloat16
SC = float(1.0 / np.sqrt(DH))


def _build():
    nc = bacc.Bacc("TRN2")
    # partition-major tiled layouts packed on host: [p, kk, i, hi/lo, free]
    x8hi = nc.dram_tensor("x8hi", [128, NKK, 2, T], FP8, kind="ExternalInput")
    x8lo = nc.dram_tensor("x8lo", [128, NKK, 2, T], FP8, kind="ExternalInput")
    wqk8 = nc.dram_tensor("wqk8", [128, NKK, 2, 2, 2 * EL], FP8, kind="ExternalInput")
    wv8 = nc.dram_tensor("wv8", [128, NKK, 2, 2, EL], FP8, kind="ExternalInput")
    woutT = nc.dram_tensor("woutT", [EL, D], BF16, kind="ExternalInput")
    mask_in = nc.dram_tensor("mask_in", [128, 128], BF16, kind="ExternalInput")
    outT = nc.dram_tensor("outT", [2, T, D], BF16, kind="ExternalOutput")

    with tile.TileContext(nc) as tc:
        with tc.tile_pool(name="pers", bufs=1) as pers, \
             tc.tile_pool(name="dram", bufs=1, space="DRAM") as dramp, \
             tc.tile_pool(name="sqp", bufs=2) as sqp, \
             tc.tile_pool(name="xlp", bufs=2) as xlp, \
             tc.tile_pool(name="qTp", bufs=2) as qTp, \
             tc.tile_pool(name="rowp", bufs=1) as rowp, \
             tc.tile_pool(name="rzrowp", bufs=2) as rzrowp, \
             tc.tile_pool(name="rzcp", bufs=2) as rzcp, \
             tc.tile_pool(name="rzbp", bufs=3) as rzbp, \
             tc.tile_pool(name="ptp", bufs=6) as ptp, \
             tc.tile_pool(name="ysbp", bufs=2) as ysbp, \
             tc.tile_pool(name="osbp", bufs=6) as osbp, \
             tc.tile_pool(name="big_ps", bufs=3, space="PSUM") as big_ps, \
             tc.tile_pool(name="st_ps", bufs=2, space="PSUM") as st_ps, \
             tc.tile_pool(name="y_ps", bufs=2, space="PSUM") as y_ps, \
             tc.tile_pool(name="red_ps", bufs=1, space="PSUM") as red_ps:

            # ---------------- persistent tiles ----------------
            xhi_sb = pers.tile([128, NKK, 2, T], FP8)
            wqk_sb = pers.tile([128, NKK, 2, 2, 2 * EL], FP8)
            wv_sb = pers.tile([128, NKK, 2, 2, EL], FP8)
            wout_sb = pers.tile([128, H_LOC, D], BF16)
            kT_sb = pers.tile([128, H_LOC, T], BF16)            # [dh, h, t]
            v_sb = pers.tile([128, NT, EL], BF16)               # [t_in_tile, j, e]
            s_col = pers.tile([128, NT], F32)                   # per-token scale / 1024
            sb_all = pers.tile([128, T], F32)                   # scale broadcast, row form
            mask_sb = pers.tile([128, 128], BF16)
            ones_sb = pers.tile([128, 8], BF16)
            bias_sb = pers.tile([128, 1], F32)
            s_scr = dramp.tile([1, T], F32)
            z_scr = dramp.tile([2 * NCH * 2, CH], F32)

            nc.gpsimd.memset(bias_sb[:], float(PS * PS * EPS))
            nc.gpsimd.memset(ones_sb[:], 1.0)

            def emit_xlo_dma(c, ring=None):
                xt = xlp.tile([128, NKK, 2, CH], FP8, tag="xlo", name=f"xlo_{c}")
                (ring or nc.sync).dma_start(xt[:], x8lo[:, :, :, c * CH:(c + 1) * CH])
                xlo_tiles[c] = xt

            # one global DMA pipe: everything rides SP (the ACT queue must
            # stay pure compute — a dma_start holds its SEQ for the whole
            # transfer).  kk-interleaved so wave A streams with arrivals.
            xlo_tiles = {}
            xlo0_t = xlp.tile([128, NKK, 2, CH], FP8, tag="xlo", name="xlo_0")
            xlo_tiles[0] = xlo0_t
            for kk in range(NKK):
                nc.sync.dma_start(wqk_sb[:, kk, :, :, :], wqk8[:, kk, :, :, :])
                nc.sync.dma_start(xhi_sb[:, kk, :, :], x8hi[:, kk, :, :])
                nc.sync.dma_start(xlo0_t[:, kk, :, :], x8lo[:, kk, :, 0:CH])
            nc.sync.dma_start(wv_sb[:, 0:4, :, :, :], wv8[:, 0:4, :, :, :])
            nc.sync.dma_start(wv_sb[:, 4:8, :, :, :], wv8[:, 4:8, :, :, :])

            # ---------------- emission helpers ----------------
            def gen_qkv_chunk(c):
                qT_c = qTp.tile([128, H_LOC, CH], BF16, tag="qT", name=f"qT_{c}")
                qT_tiles[c] = qT_c
                xlo_c = xlo_tiles[c]

                def rhs_x(kk, xl):
                    if xl == 0:
                        return xhi_sb[:, kk, :, c * CH:(c + 1) * CH]
                    return xlo_c[:, kk, :, :]

                def lhs_x(kk, xl, tt):
                    if xl == 0:
                        return xhi_sb[:, kk, :, c * CH + tt * 128:c * CH + (tt + 1) * 128]
                    return xlo_c[:, kk, :, tt * 128:(tt + 1) * 128]

                for et in list(range(4, 8)) + list(range(0, 4)):
                    ps_t = big_ps.tile([128, CH], F32, tag="big")
                    for kk in range(NKK):
                        def unit(kk=kk, et=et, ps_t=ps_t):
                            for m, (wl, xl) in enumerate(((0, 0), (0, 1), (1, 0))):
                                nc.tensor.matmul(
                                    ps_t[:], wqk_sb[:, kk, :, wl, et * 128:(et + 1) * 128],
                                    rhs_x(kk, xl), start=(kk == 0 and m == 0),
                                    stop=(kk == NKK - 1 and m == 2), perf_mode=DR)
                        yield (330, unit)
                    if et >= 4:
                        def scale(et=et, ps_t=ps_t):
                            nc.vector.tensor_tensor(kT_sb[:, et - 4, c * CH:(c + 1) * CH],
                                                    ps_t[:], sb_all[:, c * CH:(c + 1) * CH],
                                                    MULT)
                    else:
                        def scale(et=et, ps_t=ps_t, qT_c=qT_c):
                            nc.vector.tensor_tensor(qT_c[:, et, :], ps_t[:],
                                                    sb_all[:, c * CH:(c + 1) * CH], MULT)
                    yield (0, scale)
                for tt in range(QT):
                    j = c * QT + tt
                    ps_t = big_ps.tile([128, CH], F32, tag="big")
                    for kk in range(NKK):
                        def unit(kk=kk, tt=tt, ps_t=ps_t):
                            for m, (wl, xl) in enumerate(((0, 0), (0, 1), (1, 0))):
                                nc.tensor.matmul(
                                    ps_t[:], lhs_x(kk, xl, tt),
                                    wv_sb[:, kk, :, wl, :], start=(kk == 0 and m == 0),
                                    stop=(kk == NKK - 1 and m == 2), perf_mode=DR)
                        yield (330, unit)
                    def scale(j=j, ps_t=ps_t):
                        nc.vector.tensor_scalar_mul(v_sb[:, j, :], ps_t[:], s_col[:, j:j + 1])
                    yield (0, scale)

            def gen_outproj(c, h0=0, h1=H_LOC, oi=0, ring=None):
                """Yield PE micro-units for chunk c's out-projection."""
                ring = ring or nc.sync
                y_c = y_tiles[c]
                for tt in range(QT):
                    for ob in range(4):
                        o_ps = big_ps.tile([128, CH], F32, tag="big")
                        for h in range(h0, h1):
                            def unit(h=h, tt=tt, ob=ob, o_ps=o_ps, y_c=y_c):
                                nc.tensor.matmul(o_ps[:], y_c[:, h, tt * 128:(tt + 1) * 128],
                                                 wout_sb[:, h, ob * CH:(ob + 1) * CH],
                                                 start=(h == h0), stop=(h == h1 - 1))
                            yield (213, unit)
                        def copy_out(tt=tt, ob=ob, o_ps=o_ps):
                            o_sb = osbp.tile([128, CH], BF16, tag="osb")
                            nc.vector.tensor_copy(o_sb[:], o_ps[:])
                            ring.dma_start(
                                outT[oi, c * CH + tt * 128: c * CH + (tt + 1) * 128,
                                     ob * CH:(ob + 1) * CH], o_sb[:])
                        yield (0, copy_out)

            def drain(gens, ns):
                """Emit micro-units round-robin until ~ns of PE time queued."""
                acc = 0
                while gens and acc < ns:
                    try:
                        cost, fn = next(gens[0])
                        fn()
                        acc += cost
                        gens.rotate(-1)
                    except StopIteration:
                        gens.popleft()

            # ---------------- prologue: stats + chunk-0 QKV ----------------
            # squares (split ACT/DVE) + per-tile reduction tinies woven into
            # the first QKV groups so the PE never head-of-line blocks.
            ssq = red_ps.tile([128, NT], F32, tag="red", name="ssq")
            sq_tiles = []
            n = 0
            for kk in range(NKK):
                for i in range(2):
                    sq = sqp.tile([128, T], BF16, tag="sq")
                    if n % 2 == 0:
                        nc.scalar.square(sq[:], xhi_sb[:, kk, i, :])
                    else:
                        nc.vector.tensor_tensor(sq[:], xhi_sb[:, kk, i, :],
                                                xhi_sb[:, kk, i, :], MULT)
                    sq_tiles.append(sq)
                    n += 1

            def gen_stats_tinies():
                for m, sq in enumerate(sq_tiles):
                    def unit(m=m, sq=sq):
                        for tt in range(NT):
                            nc.tensor.matmul(ssq[:, tt:tt + 1], sq[:, tt * 128:(tt + 1) * 128],
                                             ones_sb[:, 0:1], start=(m == 0 and tt == 0),
                                             stop=(m == 15 and tt == NT - 1),
                                             skip_group_check=True)
                    yield (30, unit)

            qT_tiles = {}
            y_tiles = {}
            tinies = list(gen_stats_tinies())
            qT0 = qTp.tile([128, H_LOC, CH], BF16, tag="qT", name="qT_0")
            qT_tiles[0] = qT0
            xlo0 = xlo_tiles[0]

            def rhs_x0(kk, xl):
                if xl == 0:
                    return xhi_sb[:, kk, :, 0:CH]
                return xlo0[:, kk, :, :]

            # wave A: 6 q/k blocks stream kk-wise across 6 PSUM banks so the
            # PE starts with the first weight/x slices instead of waiting for
            # the full tensors.
            waveA = [4, 5, 0, 1, 6, 7]
            wa_pools = [big_ps, big_ps, big_ps, st_ps, st_ps, y_ps]
            wa_tags = ["big", "big", "big", "st", "st", "y"]
            wa_tiles = [pl.tile([128, CH], F32, tag=tg, name=f"wa_{ei}")
                        for ei, (pl, tg) in enumerate(zip(wa_pools, wa_tags))]
            for kk in range(NKK):
                for ei, et in enumerate(waveA):
                    for m, (wl, xl) in enumerate(((0, 0), (0, 1), (1, 0))):
                        nc.tensor.matmul(
                            wa_tiles[ei][:], wqk_sb[:, kk, :, wl, et * 128:(et + 1) * 128],
                            rhs_x0(kk, xl), start=(kk == 0 and m == 0),
                            stop=(kk == NKK - 1 and m == 2), perf_mode=DR)
                if kk >= 1:
                    for _, fn in tinies[2 * (kk - 1):2 * kk]:
                        fn()
            for _, fn in tinies[14:16]:
                fn()
            # stats tail: single Sqrt (one table swap), recip, transpose, bcast
            s_tmp = rowp.tile([128, NT], F32, tag="stmp")
            nc.scalar.activation(s_tmp[:], ssq[:], SQRT, bias=bias_sb[:],
                                 scale=float(PS * PS / (XS * XS * D)))
            nc.vector.reciprocal(s_col[:], s_tmp[:])
            nc.sync.dma_start(s_scr[0:1, :].rearrange("o (j p) -> p (o j)", p=128), s_col[:])
            s_row = rowp.tile([1, T], F32, tag="srow")
            nc.sync.dma_start(s_row[:], s_scr[0:1, :])
            nc.gpsimd.partition_broadcast(sb_all[:], s_row[:])
            nc.sync.dma_start(mask_sb[:], mask_in[:, :])
            emit_xlo_dma(1)
            for dl in range(H_LOC):
                nc.sync.dma_start(wout_sb[:, dl, :], woutT[dl * 128:(dl + 1) * 128, :])
            for ei, et in enumerate(waveA):
                if et >= 4:
                    nc.vector.tensor_tensor(kT_sb[:, et - 4, 0:CH], wa_tiles[ei][:],
                                            sb_all[:, 0:CH], MULT)
                else:
                    nc.vector.tensor_tensor(qT0[:, et, :], wa_tiles[ei][:],
                                            sb_all[:, 0:CH], MULT)
            # wave B: remaining q blocks + V, full-rate big_ps cycling
            for et in (2, 3):
                ps_t = big_ps.tile([128, CH], F32, tag="big")
                for kk in range(NKK):
                    for m, (wl, xl) in enumerate(((0, 0), (0, 1), (1, 0))):
                        nc.tensor.matmul(
                            ps_t[:], wqk_sb[:, kk, :, wl, et * 128:(et + 1) * 128],
                            rhs_x0(kk, xl), start=(kk == 0 and m == 0),
                            stop=(kk == NKK - 1 and m == 2), perf_mode=DR)
                nc.vector.tensor_tensor(qT0[:, et, :], ps_t[:], sb_all[:, 0:CH], MULT)
            for tt in range(QT):
                ps_t = big_ps.tile([128, CH], F32, tag="big")
                for kk in range(NKK):
                    for m, (wl, xl) in enumerate(((0, 0), (0, 1), (1, 0))):
                        if xl == 0:
                            lhs = xhi_sb[:, kk, :, tt * 128:(tt + 1) * 128]
                        else:
                            lhs = xlo0[:, kk, :, tt * 128:(tt + 1) * 128]
                        nc.tensor.matmul(ps_t[:], lhs, wv_sb[:, kk, :, wl, :],
                                         start=(kk == 0 and m == 0),
                                         stop=(kk == NKK - 1 and m == 2), perf_mode=DR)
                nc.vector.tensor_scalar_mul(v_sb[:, tt, :], ps_t[:], s_col[:, tt:tt + 1])

            # ---------------- attention phases with woven filler ----------------
            def emit_pair(c, pr, filler):
                h0, h1 = 2 * pr, 2 * pr + 1
                qT_c = qT_tiles[c]
                y_c = y_tiles[c]
                jmax = (c + 1) * QT
                yp = [y_ps.tile([128, CH], F32, tag="y", name=f"y_{c}_{pr}_{k}")
                      for k in range(2)]
                zt = red_ps.tile([128, NT], F32, tag="red", name=f"z_{c}_{pr}")
                n_z = 2 * sum(QT - (max(0, j - c * QT)) for j in range(jmax))
                i_z = 0
                for j in range(jmax):
                    off = (j - c * QT) * 128 if j >= c * QT else 0
                    sts = []
                    for k, h in ((0, h0), (1, h1)):
                        st = st_ps.tile([128, CH], F32, tag="st")
                        nc.tensor.matmul(st[:, off:], kT_sb[:, h, j * 128:(j + 1) * 128],
                                         qT_c[:, h, off:], start=True, stop=True)
                        sts.append(st)
                    drain(filler, 1500 if j >= c * QT else 1200)
                    for k, h in ((0, h0), (1, h1)):
                        pt = ptp.tile([128, CH], BF16, tag="pt")
                        nc.scalar.activation(pt[:, off:], sts[k][:, off:], EXP, scale=SC)
                        if j >= c * QT:
                            nc.gpsimd.tensor_tensor(pt[:, off:off + 128],
                                                    pt[:, off:off + 128], mask_sb[:], MULT)
                        for tt in range(off // 128, QT):
                            nc.tensor.matmul(zt[:, 4 * k + tt:4 * k + tt + 1],
                                             pt[:, tt * 128:(tt + 1) * 128], ones_sb[:, 0:1],
                                             start=(i_z == 0), stop=(i_z == n_z - 1),
                                             skip_group_check=True)
                            i_z += 1
                        nc.tensor.matmul(yp[k][:, off:], v_sb[:, j, h * 128:(h + 1) * 128],
                                         pt[:, off:], start=(j == 0), stop=(j == jmax - 1))
                rz = rzcp.tile([128, 8], F32, tag="rz")
                nc.vector.reciprocal(rz[:], zt[:, 0:8])
                slot = 2 * (c * 2 + pr)
                nc.sync.dma_start(
                    z_scr[slot:slot + 2, :].rearrange("h (j p) -> p (h j)", p=128), rz[:])
                for k, h in ((0, h0), (1, h1)):
                    rz_row = rzrowp.tile([1, CH], F32, tag="rzrow")
                    nc.sync.dma_start(rz_row[:], z_scr[slot + k:slot + k + 1, :])
                    rzb = rzbp.tile([128, CH], F32, tag="rzb")
                    nc.gpsimd.partition_broadcast(rzb[:], rz_row[:])
                    nc.vector.tensor_tensor(y_c[:, h, :], yp[k][:], rzb[:], MULT)

            filler = deque()
            qkv_gens = {}
            for c in range(NCH):
                if c + 2 < NCH:
                    emit_xlo_dma(c + 2)
                y_tiles[c] = ysbp.tile([128, H_LOC, CH], BF16, tag="ysb", name=f"ysb_{c}")
                if c + 1 < NCH:
                    qkv_gens[c + 1] = gen_qkv_chunk(c + 1)
                    filler.append(qkv_gens[c + 1])
                if c > 0:
                    filler.append(gen_outproj(c - 1))
                emit_pair(c, 0, filler)
                if c == NCH - 1:
                    filler.append(gen_outproj(c, h0=0, h1=2, oi=0, ring=nc.scalar))
                emit_pair(c, 1, filler)
                if c + 1 < NCH:
                    # chunk c+1's projections must be fully emitted before
                    # B(c+1) waits on them (in-order PE would deadlock).
                    drain(deque([qkv_gens[c + 1]]), 10 ** 9)
            drain(filler, 10 ** 9)
            drain(deque([gen_outproj(NCH - 1, h0=2, h1=H_LOC, oi=1, ring=nc.scalar)]), 10 ** 9)
    nc.finalize()
    return nc


_BUILT = None


def _get_nc():
    global _BUILT
    if _BUILT is None:
        _BUILT = _build()
    return _BUILT


def _split8(a, s):
    a = np.asarray(a, dtype=np.float32) * s
    hi = a.astype(E4NP)
    lo = (a - hi.astype(np.float32)).astype(E4NP)
    return hi, lo


def _pack(dT, s):
    hi, lo = _split8(dT, s)            # [D, width]
    a = np.stack([hi, lo], axis=1)     # [D, 2, width]
    wd = a.shape[-1]
    # [D,2,wd] -> [kk, i, p, hl, wd] -> [p, kk, i, hl, wd]
    return np.ascontiguousarray(
        a.reshape(NKK, 2, 128, 2, wd).transpose(2, 0, 1, 3, 4))


def _pack1(arr):
    # [D, wd] -> [p, kk, i, wd]
    wd = arr.shape[-1]
    return np.ascontiguousarray(arr.reshape(NKK, 2, 128, wd).transpose(2, 0, 1, 3))


def _make_in_maps(x, norm_weight, w_qkv, w_out):
    x = np.asarray(x, dtype=np.float32)
    w = np.asarray(w_qkv, dtype=np.float32) * np.asarray(norm_weight, dtype=np.float32)[None, :]
    w_out = np.asarray(w_out, dtype=np.float32)
    mask_ut = np.triu(np.ones((128, 128), dtype=np.float32)).astype(BFNP)
    in_maps = []
    for core in range(8):
        b, g = divmod(core, 4)
        sl = slice(EL * g, EL * (g + 1))
        wqk = np.concatenate([w[0 * D:1 * D][sl], w[1 * D:2 * D][sl]], axis=0)  # [1024, D]
        wv = w[2 * D:3 * D][sl]                                                 # [512, D]
        xhi, xlo = _split8(x[b].T, XS)
        in_maps.append({
            "x8hi": _pack1(xhi),
            "x8lo": _pack1(xlo),
            "wqk8": _pack(wqk.T, WS),
            "wv8": _pack(wv.T, WS),
            "woutT": np.ascontiguousarray(w_out[:, sl].T).astype(BFNP),
            "mask_in": mask_ut,
        })
    return in_maps


def _gather(results):
    out = np.zeros((B, T, D), dtype=np.float32)
    for core in range(8):
        b, _g = divmod(core, 4)
        r = results[core]["outT"].astype(np.float32)
        out[b] += r[0] + r[1]
    return out


def run(x, norm_weight, w_qkv, w_out, trace=False):
    in_maps = _make_in_maps(x, norm_weight, w_qkv, w_out)
    res = run_bass_kernel_spmd(_get_nc(), in_maps, list(range(8)), trace=False)
    return _gather(res.results), res


def kernel(x, norm_weight, w_qkv, w_out):
    out, _res = run(x, norm_weight, w_qkv, w_out)
    return out
